# revision 5
# baseline (speedup 1.0000x reference)
"""Entropic Gromov-Wasserstein distance kernel for Trainium2 (Bass/Tile).

Problem: B=2 independent GW problems, M=768, N=512, eps=0.05,
10 outer (gradient) iterations x 20 inner (Sinkhorn) iterations.

Strategy
--------
Pure data parallelism over the batch: each NeuronCore solves one full GW
problem (core c handles batch element c % 2); no collectives.

Math reformulation (validated vs the jax reference to ~2e-5 rel err):
 * Sinkhorn runs in the multiplicative domain with all a/b scale factors
   folded out analytically:  u' = 1/(K w),  w' = 1/(K^T u'), and the final
   plan is P = (1/N) * u' (*) K (*) w'.  Exponent-range analysis shows fp32
   is safe (|log K| <= 80, exp args within fp32 normal range).
 * All large matmuls exploit the symmetry of D1/D2 so no big transposes are
   ever needed:
       R^T  = P^T D1           (lhsT = P, rhs = D1)
       T3   = (R^T)^T D2       (lhsT = R^T slices, rhs = D2)
       T3^T = D2^T R^T         (lhsT = D2 slices,  rhs = R^T)
   The rank-1 terms (-0.5 * 1 t2^T etc.) are folded into the same PSUM
   accumulation as one extra contraction-1 matmul, and K = Exp(40*psum +
   per-partition bias) comes straight off the ScalarEngine.
 * Row->column relayout of the Sinkhorn vectors uses tiny PE transposes
   ((1,128) -> (128,1)) which is the cheapest cross-partition move.
 * PE matmuls run in float32r (TF32): 1 col/cycle vs 4 for fp32. Every
   tensor consumed by an fp32r matmul is produced with an fp32r-typed
   output AP (the BIR verifier requires pre-rounded operands).
"""

import sys

import numpy as np

sys.path.insert(0, "/opt/trn_rl_repo")

import concourse.bacc as bacc  # noqa: E402
import concourse.mybir as mybir  # noqa: E402
from concourse import tile  # noqa: E402

F32 = mybir.dt.float32
F32R = mybir.dt.float32r

M, N = 768, 512
MC, NC = M // 128, N // 128  # 6, 4 chunks of 128
EPS = 0.05
MAX_OUTER = 10
MAX_INNER = 20
INV_EPS = 1.0 / EPS           # 20.0
TWO_INV_EPS = 2.0 / EPS       # 40.0


def _r(ap):
    """View an fp32 AP as float32r (TF32) for/at the PE."""
    return ap.bitcast(F32R)


def build_gw_kernel(nc):
    """Emit the full GW program for one problem on one NeuronCore."""
    D1_d = nc.dram_tensor("D1", [M, M], F32, kind="ExternalInput")
    D2_d = nc.dram_tensor("D2", [N, N], F32, kind="ExternalInput")
    P_d = nc.dram_tensor("P_out", [M, N], F32, kind="ExternalOutput")
    dist_d = nc.dram_tensor("dist", [1, 1], F32, kind="ExternalOutput")

    ctx_lp = nc.allow_low_precision(reason="fp32r matmul operands (TF32 rounding)")
    ctx_lp.__enter__()
    with tile.TileContext(nc) as tc:
        with (
            tc.tile_pool(name="sb", bufs=1) as sb,
            tc.tile_pool(name="ps_bank", bufs=3, space="PSUM") as ps_bank,
            tc.tile_pool(name="ps_y", bufs=1, space="PSUM") as ps_y_pool,
            tc.tile_pool(name="ps_z", bufs=1, space="PSUM") as ps_z_pool,
            tc.tile_pool(name="ps_cols", bufs=1, space="PSUM") as ps_cols_pool,
        ):
            # ---------------- persistent SBUF tensors ----------------
            D1t = [sb.tile([128, M], F32, tag=f"D1_{c}", name=f"D1_{c}") for c in range(MC)]
            D1sq = [sb.tile([128, M], F32, tag=f"D1sq_{c}", name=f"D1sq_{c}") for c in range(MC)]
            D2t = [sb.tile([128, N], F32, tag=f"D2_{c}", name=f"D2_{c}") for c in range(NC)]
            D2sq = [sb.tile([128, N], F32, tag=f"D2sq_{c}", name=f"D2sq_{c}") for c in range(NC)]
            P = [sb.tile([128, N], F32, tag=f"P_{c}", name=f"P_{c}") for c in range(MC)]
            K = [sb.tile([128, N], F32, tag=f"K_{c}", name=f"K_{c}") for c in range(MC)]
            KT = [sb.tile([128, M], F32, tag=f"KT_{c}", name=f"KT_{c}") for c in range(NC)]
            RT = [sb.tile([128, M], F32, tag=f"RT_{c}", name=f"RT_{c}") for c in range(NC)]

            rm_cols = sb.tile([128, MC], F32, tag="rm_cols", name="rm_cols")
            cm_cols = sb.tile([128, NC], F32, tag="cm_cols", name="cm_cols")
            t1_row = sb.tile([1, M], F32, tag="t1_row", name="t1_row")     # fp32 (transposes)
            t2_row = sb.tile([1, N], F32, tag="t2_row", name="t2_row")
            t1_rowr = sb.tile([1, M], F32, tag="t1_rowr", name="t1_rowr")  # fp32r (rank-1 MM)
            t2_rowr = sb.tile([1, N], F32, tag="t2_rowr", name="t2_rowr")
            t1_bias = sb.tile([128, MC], F32, tag="t1_bias", name="t1_bias")
            t2_bias = sb.tile([128, NC], F32, tag="t2_bias", name="t2_bias")
            cm_row = sb.tile([1, N], F32, tag="cm_row", name="cm_row")

            u_row = sb.tile([1, M], F32, tag="u_row", name="u_row")
            v_row = sb.tile([1, N], F32, tag="v_row", name="v_row")
            v_rowr = sb.tile([1, N], F32, tag="v_rowr", name="v_rowr")
            U_col = sb.tile([128, MC], F32, tag="U_col", name="U_col")
            V_col = sb.tile([128, NC], F32, tag="V_col", name="V_col")
            u_scaled = sb.tile([128, MC], F32, tag="u_scaled", name="u_scaled")

            ones_col = sb.tile([128, 1], F32, tag="ones_col", name="ones_col")
            ones_row = sb.tile([1, 128], F32, tag="ones_row", name="ones_row")
            neghalf_row = sb.tile([1, 128], F32, tag="neghalf_row", name="neghalf_row")
            ident1 = sb.tile([1, 1], F32, tag="ident1", name="ident1")

            acc1 = sb.tile([128, 1], F32, tag="acc1", name="acc1")
            acc2 = sb.tile([128, 1], F32, tag="acc2", name="acc2")
            accS = sb.tile([128, NC], F32, tag="accS", name="accS")
            accSsum = sb.tile([128, 1], F32, tag="accSsum", name="accSsum")
            sc_scratch = sb.tile([128, MC], F32, tag="sc_scratch", name="sc_scratch")
            sc_scratch2 = sb.tile([128, N], F32, tag="sc_scratch2", name="sc_scratch2")
            s_row = sb.tile([1, 4], F32, tag="s_row", name="s_row")
            dist_sb = sb.tile([1, 1], F32, tag="dist_sb", name="dist_sb")

            # persistent PSUM tiles for the inner loop
            ps_y = ps_y_pool.tile([1, M], F32, tag="ps_y", name="ps_y")
            ps_z = ps_z_pool.tile([1, N], F32, tag="ps_z", name="ps_z")
            ps_ucols = ps_cols_pool.tile([128, 8], F32, tag="ps_ucols", name="ps_ucols")
            ps_vcols = ps_cols_pool.tile([128, 8], F32, tag="ps_vcols", name="ps_vcols")

            # free-dim split for fp32 moving operands (max 512 per matmul,
            # and one PSUM bank = 512 fp32)
            M_RANGES = [(0, 512), (512, 768)]
            N_RANGES = [(0, 512)]

            # ---------------- load + precompute ----------------
            for c in range(MC):
                nc.sync.dma_start(out=_r(D1t[c][:]), in_=_r(D1_d[128 * c:128 * (c + 1), :]))
                nc.scalar.square(out=_r(D1sq[c][:]), in_=D1t[c][:])
            for c in range(NC):
                nc.sync.dma_start(out=_r(D2t[c][:]), in_=_r(D2_d[128 * c:128 * (c + 1), :]))
                nc.scalar.square(out=_r(D2sq[c][:]), in_=D2t[c][:])

            # memset cannot emit float32r; stage constants via fp32 + copy
            stage1 = sb.tile([128, N], F32, tag="stage1", name="stage1")
            nc.vector.memset(stage1[:], 1.0)
            nc.vector.memset(ident1[:], 1.0)
            nc.vector.tensor_copy(_r(ones_col[:]), stage1[:, 0:1])
            nc.vector.tensor_copy(_r(ones_row[:]), stage1[0:1, 0:128])
            nc.vector.tensor_scalar_mul(_r(neghalf_row[:]), stage1[0:1, 0:128], -0.5)
            for c in range(MC):
                nc.vector.tensor_scalar_mul(_r(P[c][:]), stage1[:], 1.0 / (M * N))
            nc.vector.tensor_scalar_mul(_r(rm_cols[:]), stage1[:, 0:MC], 1.0 / M)

            def row_to_cols(row_sb, ncols, ps_tile, out_cols, round_out):
                """(1, 128*ncols) sbuf row -> (128, ncols) sbuf columns."""
                for c in range(ncols):
                    nc.tensor.matmul(
                        ps_tile[:, c:c + 1],
                        row_sb[0:1, 128 * c:128 * (c + 1)],
                        ident1[:],
                        is_transpose=True,
                        start=True,
                        stop=True,
                    )
                dst = out_cols[:, 0:ncols]
                nc.vector.tensor_copy(_r(dst) if round_out else dst,
                                      ps_tile[:, 0:ncols])

            def matvec_row(out_ps, lhs_cols, rhs_tiles, nchunk, ranges):
                """out_ps (1, F) = sum_c lhs_cols[:,c]^T @ rhs_tiles[c][:, rng]."""
                for lo, hi in ranges:
                    for c in range(nchunk):
                        nc.tensor.matmul(
                            out_ps[0:1, lo:hi],
                            _r(lhs_cols[:, c:c + 1]),
                            _r(rhs_tiles[c][:, lo:hi]),
                            start=(c == 0),
                            stop=(c == nchunk - 1),
                        )

            # ================= outer loop =================
            for outer in range(MAX_OUTER):
                # ---- column sums of P: cm = ones^T P ----
                for c in range(MC):
                    nc.tensor.matmul(
                        ps_z[0:1, 0:N],
                        _r(ones_col[:]),
                        _r(P[c][:]),
                        start=(c == 0),
                        stop=(c == MC - 1),
                    )
                nc.scalar.copy(out=cm_row[:], in_=ps_z[0:1, 0:N])
                row_to_cols(cm_row, NC, ps_vcols, cm_cols, round_out=True)

                # ---- t1 = D1sq @ rm ; t2 = D2sq @ cm ----
                matvec_row(ps_y, rm_cols, D1sq, MC, M_RANGES)
                nc.scalar.copy(out=t1_row[:], in_=ps_y[0:1, 0:M])
                nc.vector.tensor_copy(_r(t1_rowr[:]), ps_y[0:1, 0:M])
                row_to_cols(t1_row, MC, ps_ucols, t1_bias, round_out=False)
                nc.vector.tensor_scalar_mul(t1_bias[:], t1_bias[:], -INV_EPS)

                matvec_row(ps_z, cm_cols, D2sq, NC, N_RANGES)
                nc.scalar.copy(out=t2_row[:], in_=ps_z[0:1, 0:N])
                nc.vector.tensor_copy(_r(t2_rowr[:]), ps_z[0:1, 0:N])
                row_to_cols(t2_row, NC, ps_vcols, t2_bias, round_out=False)
                nc.vector.tensor_scalar_mul(t2_bias[:], t2_bias[:], -INV_EPS)

                # ---- R^T = P^T D1  (N-part, M-free) ----
                for j in range(NC):
                    for lo, hi in M_RANGES:
                        ps = ps_bank.tile([128, 512], F32, tag="ps_work", name="ps_rt")
                        for c in range(MC):
                            nc.tensor.matmul(
                                ps[:, 0:hi - lo],
                                _r(P[c][:, 128 * j:128 * (j + 1)]),
                                _r(D1t[c][:, lo:hi]),
                                start=(c == 0),
                                stop=(c == MC - 1),
                            )
                        nc.vector.tensor_copy(_r(RT[j][:, lo:hi]), ps[:, 0:hi - lo])

                # ---- K^T: T3'^T = D2 R^T - 0.5 1 t1^T ; KT = Exp(40x + bias) ----
                for n in range(NC):
                    for lo, hi in M_RANGES:
                        ps = ps_bank.tile([128, 512], F32, tag="ps_work", name="ps_t3t")
                        for j in range(NC):
                            nc.tensor.matmul(
                                ps[:, 0:hi - lo],
                                _r(D2t[j][:, 128 * n:128 * (n + 1)]),
                                _r(RT[j][:, lo:hi]),
                                start=(j == 0),
                                stop=False,
                            )
                        nc.tensor.matmul(
                            ps[:, 0:hi - lo],
                            _r(neghalf_row[:]),
                            _r(t1_rowr[0:1, lo:hi]),
                            start=False,
                            stop=True,
                        )
                        nc.scalar.activation(
                            _r(KT[n][:, lo:hi]),
                            ps[:, 0:hi - lo],
                            mybir.ActivationFunctionType.Exp,
                            bias=t2_bias[:, n:n + 1],
                            scale=TWO_INV_EPS,
                        )

                # ---- K: T3' = (R^T)^T D2 - 0.5 1 t2^T ; K = Exp(40x + bias) ----
                for i in range(MC):
                    ps = ps_bank.tile([128, 512], F32, tag="ps_work", name="ps_t3")
                    for j in range(NC):
                        nc.tensor.matmul(
                            ps[:, 0:N],
                            _r(RT[j][:, 128 * i:128 * (i + 1)]),
                            _r(D2t[j][:]),
                            start=(j == 0),
                            stop=False,
                        )
                    nc.tensor.matmul(
                        ps[:, 0:N],
                        _r(neghalf_row[:]),
                        _r(t2_rowr[0:1, 0:N]),
                        start=False,
                        stop=True,
                    )
                    nc.scalar.activation(
                        _r(K[i][:]),
                        ps[:, 0:N],
                        mybir.ActivationFunctionType.Exp,
                        bias=t1_bias[:, i:i + 1],
                        scale=TWO_INV_EPS,
                    )

                # ---- Sinkhorn inner loop (multiplicative, unscaled) ----
                nc.vector.tensor_copy(_r(V_col[:]), stage1[:, 0:NC])
                for it in range(MAX_INNER):
                    matvec_row(ps_y, V_col, KT, NC, M_RANGES)
                    nc.vector.reciprocal(u_row[:], ps_y[0:1, 0:M])
                    row_to_cols(u_row, MC, ps_ucols, U_col, round_out=True)
                    matvec_row(ps_z, U_col, K, MC, N_RANGES)
                    nc.vector.reciprocal(v_row[:], ps_z[0:1, 0:N])
                    row_to_cols(v_row, NC, ps_vcols, V_col, round_out=True)

                # ---- P = (1/N) u' (*) K (*) w'  (+ row sums) ----
                nc.vector.tensor_copy(_r(v_rowr[:]), v_row[:])
                ps_vb = ps_bank.tile([128, 512], F32, tag="ps_work", name="ps_vb")
                nc.tensor.matmul(
                    ps_vb[:, 0:N],
                    _r(ones_row[:]),
                    _r(v_rowr[0:1, 0:N]),
                    start=True,
                    stop=True,
                )
                nc.vector.tensor_scalar_mul(u_scaled[:], U_col[:], 1.0 / N)
                for i in range(MC):
                    nc.vector.scalar_tensor_tensor(
                        out=_r(P[i][:]),
                        in0=K[i][:],
                        scalar=u_scaled[:, i:i + 1],
                        in1=ps_vb[:, 0:N],
                        op0=mybir.AluOpType.mult,
                        op1=mybir.AluOpType.mult,
                        accum_out=_r(rm_cols[:, i:i + 1]),
                    )

            # ================= outputs =================
            for i in range(MC):
                nc.sync.dma_start(out=P_d[128 * i:128 * (i + 1), :], in_=P[i][:])

            # final column sums
            for c in range(MC):
                nc.tensor.matmul(
                    ps_z[0:1, 0:N],
                    _r(ones_col[:]),
                    _r(P[c][:]),
                    start=(c == 0),
                    stop=(c == MC - 1),
                )
            nc.scalar.copy(out=cm_row[:], in_=ps_z[0:1, 0:N])
            row_to_cols(cm_row, NC, ps_vcols, cm_cols, round_out=True)

            # t1s = rm . (D1sq rm)
            matvec_row(ps_y, rm_cols, D1sq, MC, M_RANGES)
            nc.scalar.copy(out=t1_row[:], in_=ps_y[0:1, 0:M])
            row_to_cols(t1_row, MC, ps_ucols, t1_bias, round_out=False)
            nc.vector.scalar_tensor_tensor(
                out=sc_scratch[:],
                in0=t1_bias[:, 0:MC],
                scalar=1.0,
                in1=rm_cols[:],
                op0=mybir.AluOpType.mult,
                op1=mybir.AluOpType.mult,
                accum_out=_r(acc1[:]),
            )
            # t2s = cm . (D2sq cm)
            matvec_row(ps_z, cm_cols, D2sq, NC, N_RANGES)
            nc.scalar.copy(out=t2_row[:], in_=ps_z[0:1, 0:N])
            row_to_cols(t2_row, NC, ps_vcols, t2_bias, round_out=False)
            nc.vector.scalar_tensor_tensor(
                out=sc_scratch[:, 0:NC],
                in0=t2_bias[:, 0:NC],
                scalar=1.0,
                in1=cm_cols[:],
                op0=mybir.AluOpType.mult,
                op1=mybir.AluOpType.mult,
                accum_out=_r(acc2[:]),
            )

            # cross = sum( (R_f^T P) (*) D2 ), R_f = D1 P (stored into K tiles)
            for i in range(MC):
                ps = ps_bank.tile([128, 512], F32, tag="ps_work", name="ps_rf")
                for c in range(MC):
                    nc.tensor.matmul(
                        ps[:, 0:N],
                        _r(D1t[c][:, 128 * i:128 * (i + 1)]),
                        _r(P[c][:]),
                        start=(c == 0),
                        stop=(c == MC - 1),
                    )
                nc.vector.tensor_copy(_r(K[i][:]), ps[:, 0:N])
            for j in range(NC):
                ps = ps_bank.tile([128, 512], F32, tag="ps_work", name="ps_s")
                for c in range(MC):
                    nc.tensor.matmul(
                        ps[:, 0:N],
                        _r(K[c][:, 128 * j:128 * (j + 1)]),
                        _r(P[c][:]),
                        start=(c == 0),
                        stop=(c == MC - 1),
                    )
                nc.vector.scalar_tensor_tensor(
                    out=sc_scratch2[:],
                    in0=ps[:, 0:N],
                    scalar=1.0,
                    in1=D2t[j][:],
                    op0=mybir.AluOpType.mult,
                    op1=mybir.AluOpType.mult,
                    accum_out=accS[:, j:j + 1],
                )
            nc.vector.tensor_reduce(
                _r(accSsum[:]), accS[:], axis=mybir.AxisListType.X,
                op=mybir.AluOpType.add,
            )

            # partition-sum the three accumulators via ones matvec
            for k, acc in enumerate((acc1, acc2, accSsum)):
                # fp32 here: fp32r matmul dst patterns disallow 1-wide outputs
                nc.tensor.matmul(
                    ps_z[0:1, k:k + 1],
                    acc[:],
                    ones_col[:],
                    start=True,
                    stop=True,
                )
            nc.scalar.copy(out=s_row[0:1, 0:3], in_=ps_z[0:1, 0:3])
            # dist = t1s + t2s - 2*cross
            nc.vector.tensor_tensor(
                out=dist_sb[:],
                in0=s_row[0:1, 0:1],
                in1=s_row[0:1, 1:2],
                op=mybir.AluOpType.add,
            )
            nc.vector.scalar_tensor_tensor(
                out=dist_sb[:],
                in0=s_row[0:1, 2:3],
                scalar=-2.0,
                in1=dist_sb[:],
                op0=mybir.AluOpType.mult,
                op1=mybir.AluOpType.add,
            )
            nc.sync.dma_start(out=dist_d[:], in_=dist_sb[:])

    ctx_lp.__exit__(None, None, None)
    return nc


_CACHED_NC = None


def get_nc():
    global _CACHED_NC
    if _CACHED_NC is None:
        nc = bacc.Bacc("TRN2", target_bir_lowering=False, debug=False)
        build_gw_kernel(nc)
        nc.compile()
        _CACHED_NC = nc
    return _CACHED_NC


N_CORES = 8


def run_on_hw(D1, D2, trace=False):
    """Run the SPMD kernel; returns (results_list, BassKernelResults)."""
    from concourse.bass_utils import run_bass_kernel_spmd

    nc = get_nc()
    B = D1.shape[0]
    in_maps = [
        {"D1": np.ascontiguousarray(D1[c % B]), "D2": np.ascontiguousarray(D2[c % B])}
        for c in range(N_CORES)
    ]
    res = run_bass_kernel_spmd(
        nc, in_maps, core_ids=list(range(N_CORES)), trace=trace,
        trace_cores=list(range(N_CORES)) if trace else None,
    )
    return res.results, res


def kernel(D1, D2):
    D1 = np.asarray(D1, np.float32)
    D2 = np.asarray(D2, np.float32)
    B = D1.shape[0]
    results, _ = run_on_hw(D1, D2, trace=False)
    dist = np.array([results[b]["dist"][0, 0] for b in range(B)], np.float32)
    P = np.stack([results[b]["P_out"] for b in range(B)]).astype(np.float32)
    return dist, P
